# revision 19
# baseline (speedup 1.0000x reference)
"""Trainium2 Bass kernel for DifferentiableDefocusRenderer — v3.

Math (mirrors the reference):
  planes = linspace(0, 50, 32); per-plane depthwise Gaussian blur of
  sharp_image (separable, kernel k<=31, truncated+renormalized), output =
  per-pixel hard select of the blurred plane by CoC bucket.

Distribution: pure data parallel, 8 cores = (batch b in 0..3) x (H half).
Each core computes [3, 256, 512] of output for its (b, half).

Per-core pipeline (all-plane dense, bf16 matmuls):
  pass A (column conv, 8-plane quarters, M-packed):
      C[x, i, y] = sum_k X[k, x] * T1[k, (i,q)]   (role-swapped matmul:
      stationary = X y-window tile, moving = multi-plane Toeplitz T1)
  pass B (row conv, role-swapped so output lands [y, x] directly):
      stationary = C chunk [x-slice 128, y-block 128], moving = combined
      Toeplitz TF[x-in, i, j] streaming 602 cols/(plane, yblk); boundary
      strips accumulate across slice-adjacent matmuls (start=False).
  select: copy_predicated(acc[y, x], mask_i, pb) with one-hot plane masks.
  No final transpose: acc is already [y, x]; DMA straight out.
C quarters are double-buffered so pass A(q+1) overlaps pass B(q).
"""

import sys

import numpy as np
import ml_dtypes

sys.path.insert(0, "/opt/trn_rl_repo")

B, C, H, W = 4, 3, 512, 512
MAX_COC = 50.0
NPLANES = 32
HALF = 256          # output rows per core
YT = 64             # output rows per pass-A y-tile
NT = HALF // YT     # 4 y-tiles
NS = W // 128       # 4 x slices
NQ = 4              # plane quarters (8 planes each)
BF16 = ml_dtypes.bfloat16

_CACHE = {}


# ----------------------------------------------------------------------------
# host-side tables (exactly mirroring reference kernel construction)
# ----------------------------------------------------------------------------

def _gaussian_kernel_1d(coc_value):
    # mirrors reference._gaussian_kernel_np (1-D factor of the outer product)
    sigma = coc_value / 2.355
    k = int(2 * coc_value + 1)
    if k % 2 == 0:
        k += 1
    k = min(k, 31)
    coords = np.arange(k, dtype=np.float32) - (k // 2)
    g = np.exp(-coords ** 2 / (2.0 * sigma ** 2))
    g = g / g.sum()
    return g.astype(np.float32)  # [k]


def _plane_kernels():
    """g31[i] in R^31, centered; plane 0 = identity delta."""
    planes = np.linspace(0.0, MAX_COC, NPLANES, dtype=np.float32)
    g31 = np.zeros((NPLANES, 31), dtype=np.float32)
    for i in range(NPLANES):
        coc = float(planes[i])
        if coc < 0.5:
            g31[i, 15] = 1.0
        else:
            g = _gaussian_kernel_1d(coc)
            k = g.shape[0]
            off = (31 - k) // 2
            g31[i, off:off + k] = g
    return planes, g31


def _host_tables():
    planes, g31 = _plane_kernels()

    # T1[k, q4, il*64 + q] = g31[8*q4 + il][k - q - 17]
    t1 = np.zeros((128, NQ, 512), dtype=np.float32)
    for q4 in range(NQ):
        for il in range(8):
            for q in range(64):
                for k in range(max(0, q + 17), min(128, q + 48)):
                    t1[k, q4, il * 64 + q] = g31[8 * q4 + il, k - q - 17]

    # Combined row-conv Toeplitz for pass B (C chunk stationary):
    # TF[k, i, j] = g31[i][k - j + 30], valid when 0 <= k-j+30 <= 30.
    # Column j maps to output xo = 128*s + j - 15 for slice s.
    tf = np.zeros((128, NPLANES, 158), dtype=np.float32)
    for j in range(158):
        for k in range(max(0, j - 30), min(128, j + 1)):
            tf[k, :, j] = g31[:, k - j + 30]

    return planes, t1.astype(BF16), tf.astype(BF16)


def _plane_index(coc):
    """Exact bucket index per pixel, replicating reference fp32 comparisons."""
    planes = np.linspace(0.0, MAX_COC, NPLANES, dtype=np.float32)
    bnd = ((planes[:-1] + planes[1:]) / np.float32(2.0)).astype(np.float32)
    coc = coc.astype(np.float32)
    p = np.zeros(coc.shape, dtype=np.int32)
    for i in range(NPLANES - 1):
        p += (coc > bnd[i]).astype(np.int32)
    return p  # [H, W] int in [0, 31]


# ----------------------------------------------------------------------------
# device program
# ----------------------------------------------------------------------------

def _build_program():
    import concourse.bacc as bacc
    import concourse.mybir as mybir
    import concourse.tile as tile

    dt = mybir.dt
    nc = bacc.Bacc("TRN2", target_bir_lowering=False,
                   debug=False, enable_asserts=False, num_devices=8)

    xin_d = nc.dram_tensor("xin", [C, NT, 128, 512], dt.bfloat16,
                           kind="ExternalInput")
    t1_d = nc.dram_tensor("t1", [128, NQ, 512], dt.bfloat16,
                          kind="ExternalInput")
    tf_d = nc.dram_tensor("tf", [128, NPLANES, 158], dt.bfloat16,
                          kind="ExternalInput")
    mk_d = nc.dram_tensor("mk", [128, NPLANES, 2, 512], dt.uint8,
                          kind="ExternalInput")
    out_d = nc.dram_tensor("out", [C, 2, 128, 512], dt.float32,
                           kind="ExternalOutput")

    with tile.TileContext(nc) as tc:
        with (
            tc.tile_pool(name="const", bufs=1) as const_pool,
            tc.tile_pool(name="cbuf", bufs=2) as c_pool,
            tc.tile_pool(name="xin", bufs=2) as x_pool,
            tc.tile_pool(name="accp", bufs=2) as acc_pool,
            tc.tile_pool(name="psA", bufs=3, space="PSUM") as psA,
            tc.tile_pool(name="psB", bufs=4, space="PSUM") as psB,
        ):
            # ---- constants ----
            t1_s = const_pool.tile([128, NQ, 512], dt.bfloat16, tag="t1",
                                   name="t1_s")
            nc.sync.dma_start(t1_s[:], t1_d.ap()[:])
            tf_s = const_pool.tile([128, NPLANES, 158], dt.bfloat16,
                                   tag="tf", name="tf_s")
            nc.sync.dma_start(tf_s[:], tf_d.ap()[:])
            mk_s = const_pool.tile([128, NPLANES, 2, 512], dt.uint8,
                                   tag="mk", name="mk_s")
            nc.sync.dma_start(mk_s[:], mk_d.ap()[:])

            for ch in range(C):
                acc = acc_pool.tile([128, 2, 512], dt.float32,
                                    tag="acc", name="acc")

                xts = []
                for t in range(NT):
                    xt = x_pool.tile([128, 512], dt.bfloat16, tag=f"xt{t}",
                                     name=f"xt{t}")
                    nc.sync.dma_start(xt[:], xin_d.ap()[ch, t])
                    xts.append(xt)

                for q4 in range(NQ):
                    # ---- pass A quarter: C[x, s, il, y] ----
                    # Emitted in y-block halves so pass B for yb can start
                    # after only half of pass A (engines run program order).
                    c_all = c_pool.tile([128, NS, 8, HALF], dt.bfloat16,
                                        tag="c", name="c_all")
                    for yb in range(2):
                        for t in (2 * yb, 2 * yb + 1):
                            for s in range(NS):
                                pa = psA.tile([128, 512], dt.float32,
                                              tag="pa", name="pa")
                                nc.tensor.matmul(
                                    pa[:], xts[t][:, 128 * s:128 * (s + 1)],
                                    t1_s[:, q4, :], start=True, stop=True)
                                # psum [x, (i8,q64)] -> C[x, il, 64t+q]
                                y0 = YT * t
                                nc.scalar.copy(
                                    c_all[:, s, :, y0:y0 + YT],
                                    pa.rearrange("p (i q) -> p i q", i=8))

                        # ---- pass B + select for this quarter/y-block ----
                        # out[y, xo] = sum_k C[k, y] * TF[k, i, xo-128s+15]
                        for il in range(8):
                            i = 8 * q4 + il
                            pb = psB.tile([128, 512], dt.float32,
                                          tag="pb", name="pb")
                            yy = slice(128 * yb, 128 * (yb + 1))
                            # Accumulation groups must be (T,F) ... (F,T)
                            # with nothing else to the bank in between:
                            # open slice s's fresh region, close with slice
                            # s+1's boundary-strip accumulate.
                            nc.tensor.matmul(          # fresh [0, 143)
                                pb[:, 0:143], c_all[:, 0, il, yy],
                                tf_s[:, i, 15:158],
                                start=True, stop=False,
                                skip_group_check=True)
                            for s in (1, 2, 3):
                                x0 = 128 * s
                                nc.tensor.matmul(      # strip [x0-15, x0+15)
                                    pb[:, x0 - 15:x0 + 15],
                                    c_all[:, s, il, yy],
                                    tf_s[:, i, 0:30],
                                    start=False, stop=True,
                                    skip_group_check=True)
                                hi = 143 if s < 3 else 128
                                nc.tensor.matmul(      # fresh interior
                                    pb[:, x0 + 15:x0 + hi],
                                    c_all[:, s, il, yy],
                                    tf_s[:, i, 30:15 + hi],
                                    start=True, stop=(s == 3),
                                    skip_group_check=True)
                            nc.vector.copy_predicated(
                                acc[:, yb, :], mk_s[:, i, yb, :], pb[:])

                # ---- store (already [y, x]) ----
                for u in range(2):
                    nc.sync.dma_start(out_d.ap()[ch, u], acc[:, u, :])

    nc.compile()
    return nc


# ----------------------------------------------------------------------------
# host orchestration
# ----------------------------------------------------------------------------

def _prepare_in_maps(sharp_image, coc_map):
    planes, t1, tf = _CACHE["tables"]
    p_full = {}
    in_maps = []
    for core in range(8):
        b, h = divmod(core, 2)
        y0 = HALF * h
        # X padded rows [-32, 288) local
        xpad = np.zeros((C, HALF + 64, W), dtype=BF16)
        glo = y0 - 32
        ghi = y0 + HALF + 32
        clo, chi = max(0, glo), min(H, ghi)
        xpad[:, clo - glo:chi - glo, :] = sharp_image[b, :, clo:chi, :]
        xin = np.zeros((C, NT, 128, W), dtype=BF16)
        for t in range(NT):
            xin[:, t] = xpad[:, YT * t:YT * t + 128, :]

        if b not in p_full:
            p_full[b] = _plane_index(coc_map[b, 0])
        p = p_full[b][y0:y0 + HALF, :]  # [HALF, W]
        # one-hot masks: mk[m, i, yb, x] = (p[128*yb + m, x] == i)
        pr = p.reshape(2, 128, W).transpose(1, 0, 2)  # [128, 2, 512]
        mk = (pr[:, None, :, :] ==
              np.arange(NPLANES, dtype=np.int32)[None, :, None, None]
              ).astype(np.uint8)

        in_maps.append({
            "xin": xin,
            "t1": t1, "tf": tf,
            "mk": mk,
        })
    return in_maps


def _assemble(results):
    out = np.zeros((B, C, H, W), dtype=np.float32)
    for core in range(8):
        b, h = divmod(core, 2)
        r = results[core]["out"]  # [C, 2, 128, 512]
        out[b, :, HALF * h:HALF * (h + 1), :] = r.reshape(C, HALF, W)
    return out


def run(inputs, trace=False):
    from concourse import bass_utils
    if "tables" not in _CACHE:
        _CACHE["tables"] = _host_tables()
    if "nc" not in _CACHE:
        _CACHE["nc"] = _build_program()
    nc = _CACHE["nc"]
    in_maps = _prepare_in_maps(inputs["sharp_image"], inputs["coc_map"])
    res = bass_utils.run_bass_kernel_spmd(
        nc, in_maps, core_ids=list(range(8)), trace=trace)
    return _assemble(res.results), res


def kernel(**inputs):
    out, _ = run(inputs)
    return out


# revision 20
# speedup vs baseline: 1.1632x; 1.1632x over previous
"""Trainium2 Bass kernel for DifferentiableDefocusRenderer — v3.

Math (mirrors the reference):
  planes = linspace(0, 50, 32); per-plane depthwise Gaussian blur of
  sharp_image (separable, kernel k<=31, truncated+renormalized), output =
  per-pixel hard select of the blurred plane by CoC bucket.

Distribution: pure data parallel, 8 cores = (batch b in 0..3) x (H half).
Each core computes [3, 256, 512] of output for its (b, half).

Per-core pipeline (all-plane dense, bf16 matmuls):
  pass A (column conv, 8-plane quarters, M-packed):
      C[x, i, y] = sum_k X[k, x] * T1[k, (i,q)]   (role-swapped matmul:
      stationary = X y-window tile, moving = multi-plane Toeplitz T1)
  pass B (row conv, role-swapped so output lands [y, x] directly):
      stationary = C chunk [x-slice 128, y-block 128], moving = combined
      Toeplitz TF[x-in, i, j] streaming 602 cols/(plane, yblk); boundary
      strips accumulate across slice-adjacent matmuls (start=False).
  select: copy_predicated(acc[y, x], mask_i, pb) with one-hot plane masks.
  No final transpose: acc is already [y, x]; DMA straight out.
C quarters are double-buffered so pass A(q+1) overlaps pass B(q).
"""

import sys

import numpy as np
import ml_dtypes

sys.path.insert(0, "/opt/trn_rl_repo")

B, C, H, W = 4, 3, 512, 512
MAX_COC = 50.0
NPLANES = 32
HALF = 256          # output rows per core
YT = 64             # output rows per pass-A y-tile
NT = HALF // YT     # 4 y-tiles
NS = W // 128       # 4 x slices
NQ = 4              # plane quarters (8 planes each)
BF16 = ml_dtypes.bfloat16

_CACHE = {}


# ----------------------------------------------------------------------------
# host-side tables (exactly mirroring reference kernel construction)
# ----------------------------------------------------------------------------

def _gaussian_kernel_1d(coc_value):
    # mirrors reference._gaussian_kernel_np (1-D factor of the outer product)
    sigma = coc_value / 2.355
    k = int(2 * coc_value + 1)
    if k % 2 == 0:
        k += 1
    k = min(k, 31)
    coords = np.arange(k, dtype=np.float32) - (k // 2)
    g = np.exp(-coords ** 2 / (2.0 * sigma ** 2))
    g = g / g.sum()
    return g.astype(np.float32)  # [k]


def _plane_kernels():
    """g31[i] in R^31, centered; plane 0 = identity delta."""
    planes = np.linspace(0.0, MAX_COC, NPLANES, dtype=np.float32)
    g31 = np.zeros((NPLANES, 31), dtype=np.float32)
    for i in range(NPLANES):
        coc = float(planes[i])
        if coc < 0.5:
            g31[i, 15] = 1.0
        else:
            g = _gaussian_kernel_1d(coc)
            k = g.shape[0]
            off = (31 - k) // 2
            g31[i, off:off + k] = g
    return planes, g31


def _host_tables():
    planes, g31 = _plane_kernels()

    # T1[k, q4, il*64 + q] = g31[8*q4 + il][k - q - 17]
    t1 = np.zeros((128, NQ, 512), dtype=np.float32)
    for q4 in range(NQ):
        for il in range(8):
            for q in range(64):
                for k in range(max(0, q + 17), min(128, q + 48)):
                    t1[k, q4, il * 64 + q] = g31[8 * q4 + il, k - q - 17]

    # Combined row-conv Toeplitz for pass B (C chunk stationary):
    # TF[k, i, j] = g31[i][k - j + 30], valid when 0 <= k-j+30 <= 30.
    # Column j maps to output xo = 128*s + j - 15 for slice s.
    tf = np.zeros((128, NPLANES, 158), dtype=np.float32)
    for j in range(158):
        for k in range(max(0, j - 30), min(128, j + 1)):
            tf[k, :, j] = g31[:, k - j + 30]

    return planes, t1.astype(BF16), tf.astype(BF16)


def _plane_index(coc):
    """Exact bucket index per pixel, replicating reference fp32 comparisons."""
    planes = np.linspace(0.0, MAX_COC, NPLANES, dtype=np.float32)
    bnd = ((planes[:-1] + planes[1:]) / np.float32(2.0)).astype(np.float32)
    coc = coc.astype(np.float32)
    p = np.zeros(coc.shape, dtype=np.int32)
    for i in range(NPLANES - 1):
        p += (coc > bnd[i]).astype(np.int32)
    return p  # [H, W] int in [0, 31]


# ----------------------------------------------------------------------------
# device program
# ----------------------------------------------------------------------------

def _build_program():
    import concourse.bacc as bacc
    import concourse.mybir as mybir
    import concourse.tile as tile

    dt = mybir.dt
    nc = bacc.Bacc("TRN2", target_bir_lowering=False,
                   debug=False, enable_asserts=False, num_devices=8)

    xin_d = nc.dram_tensor("xin", [C, NT, 128, 512], dt.bfloat16,
                           kind="ExternalInput")
    t1_d = nc.dram_tensor("t1", [128, NQ, 512], dt.bfloat16,
                          kind="ExternalInput")
    tf_d = nc.dram_tensor("tf", [128, NPLANES, 158], dt.bfloat16,
                          kind="ExternalInput")
    pmap_d = nc.dram_tensor("pmap", [128, 2, 512], dt.bfloat16,
                            kind="ExternalInput")
    out_d = nc.dram_tensor("out", [C, 2, 128, 512], dt.float32,
                           kind="ExternalOutput")

    with tile.TileContext(nc) as tc:
        with (
            tc.tile_pool(name="const", bufs=1) as const_pool,
            tc.tile_pool(name="cbuf", bufs=2) as c_pool,
            tc.tile_pool(name="xin", bufs=2) as x_pool,
            tc.tile_pool(name="accp", bufs=2) as acc_pool,
            tc.tile_pool(name="psA", bufs=3, space="PSUM") as psA,
            tc.tile_pool(name="psB", bufs=4, space="PSUM") as psB,
        ):
            # ---- constants ----
            t1_s = const_pool.tile([128, NQ, 512], dt.bfloat16, tag="t1",
                                   name="t1_s")
            nc.sync.dma_start(t1_s[:], t1_d.ap()[:])
            tf_s = const_pool.tile([128, NPLANES, 158], dt.bfloat16,
                                   tag="tf", name="tf_s")
            nc.sync.dma_start(tf_s[:], tf_d.ap()[:])
            pmap_s = const_pool.tile([128, 2, 512], dt.bfloat16, tag="pmap",
                                     name="pmap_s")
            nc.sync.dma_start(pmap_s[:], pmap_d.ap()[:])

            masks = {}

            for ch in range(C):
                acc = acc_pool.tile([128, 2, 512], dt.float32,
                                    tag="acc", name="acc")

                xts = []
                for t in range(NT):
                    xt = x_pool.tile([128, 512], dt.bfloat16, tag=f"xt{t}",
                                     name=f"xt{t}")
                    nc.sync.dma_start(xt[:], xin_d.ap()[ch, t])
                    xts.append(xt)

                for q4 in range(NQ):
                    # ---- pass A quarter: C[x, s, il, y] ----
                    c_all = c_pool.tile([128, NS, 8, HALF], dt.bfloat16,
                                        tag="c", name="c_all")
                    for t in range(NT):
                        for s in range(NS):
                            pa = psA.tile([128, 512], dt.float32, tag="pa",
                                          name="pa")
                            nc.tensor.matmul(
                                pa[:], xts[t][:, 128 * s:128 * (s + 1)],
                                t1_s[:, q4, :], start=True, stop=True)
                            # psum [x, (i8,q64)] -> C[x, il, 64t+q]
                            y0 = YT * t
                            nc.scalar.copy(
                                c_all[:, s, :, y0:y0 + YT],
                                pa.rearrange("p (i q) -> p i q", i=8))

                    if ch == 0 and q4 == 0:
                        # one-hot plane masks (shared across channels)
                        for i in range(NPLANES):
                            mk = const_pool.tile([128, 2, 512], dt.uint8,
                                                 tag=f"mask{i}",
                                                 name=f"mask{i}")
                            nc.vector.tensor_scalar(
                                mk[:], pmap_s[:],
                                float(i), None,
                                mybir.AluOpType.is_equal)
                            masks[i] = mk

                    # ---- pass B + select for this quarter ----
                    # out[y, xo] = sum_k C[k, y] * TF[k, i, xo-128s+15]
                    for il in range(8):
                        i = 8 * q4 + il
                        for yb in range(2):
                            pb = psB.tile([128, 512], dt.float32,
                                          tag="pb", name="pb")
                            yy = slice(128 * yb, 128 * (yb + 1))
                            # Accumulation groups must be (T,F) ... (F,T)
                            # with nothing else to the bank in between:
                            # open slice s's fresh region, close with slice
                            # s+1's boundary-strip accumulate.
                            nc.tensor.matmul(          # fresh [0, 143)
                                pb[:, 0:143], c_all[:, 0, il, yy],
                                tf_s[:, i, 15:158],
                                start=True, stop=False,
                                skip_group_check=True)
                            for s in (1, 2, 3):
                                x0 = 128 * s
                                nc.tensor.matmul(      # strip [x0-15, x0+15)
                                    pb[:, x0 - 15:x0 + 15],
                                    c_all[:, s, il, yy],
                                    tf_s[:, i, 0:30],
                                    start=False, stop=True,
                                    skip_group_check=True)
                                hi = 143 if s < 3 else 128
                                nc.tensor.matmul(      # fresh interior
                                    pb[:, x0 + 15:x0 + hi],
                                    c_all[:, s, il, yy],
                                    tf_s[:, i, 30:15 + hi],
                                    start=True, stop=(s == 3),
                                    skip_group_check=True)
                            nc.vector.copy_predicated(
                                acc[:, yb, :], masks[i][:, yb, :], pb[:])

                # ---- store (already [y, x]) ----
                for u in range(2):
                    nc.sync.dma_start(out_d.ap()[ch, u], acc[:, u, :])

    nc.compile()
    return nc


# ----------------------------------------------------------------------------
# host orchestration
# ----------------------------------------------------------------------------

def _prepare_in_maps(sharp_image, coc_map):
    planes, t1, tf = _CACHE["tables"]
    p_full = {}
    in_maps = []
    for core in range(8):
        b, h = divmod(core, 2)
        y0 = HALF * h
        # X padded rows [-32, 288) local
        xpad = np.zeros((C, HALF + 64, W), dtype=BF16)
        glo = y0 - 32
        ghi = y0 + HALF + 32
        clo, chi = max(0, glo), min(H, ghi)
        xpad[:, clo - glo:chi - glo, :] = sharp_image[b, :, clo:chi, :]
        xin = np.zeros((C, NT, 128, W), dtype=BF16)
        for t in range(NT):
            xin[:, t] = xpad[:, YT * t:YT * t + 128, :]

        if b not in p_full:
            p_full[b] = _plane_index(coc_map[b, 0])
        p = p_full[b][y0:y0 + HALF, :]  # [HALF, W]
        # pmap[m, yb, x] = p[128*yb + m, x]
        pmap = np.ascontiguousarray(
            p.reshape(2, 128, W).transpose(1, 0, 2)).astype(BF16)

        in_maps.append({
            "xin": xin,
            "t1": t1, "tf": tf,
            "pmap": pmap,
        })
    return in_maps


def _assemble(results):
    out = np.zeros((B, C, H, W), dtype=np.float32)
    for core in range(8):
        b, h = divmod(core, 2)
        r = results[core]["out"]  # [C, 2, 128, 512]
        out[b, :, HALF * h:HALF * (h + 1), :] = r.reshape(C, HALF, W)
    return out


def run(inputs, trace=False):
    from concourse import bass_utils
    if "tables" not in _CACHE:
        _CACHE["tables"] = _host_tables()
    if "nc" not in _CACHE:
        _CACHE["nc"] = _build_program()
    nc = _CACHE["nc"]
    in_maps = _prepare_in_maps(inputs["sharp_image"], inputs["coc_map"])
    res = bass_utils.run_bass_kernel_spmd(
        nc, in_maps, core_ids=list(range(8)), trace=trace)
    return _assemble(res.results), res


def kernel(**inputs):
    out, _ = run(inputs)
    return out
